# revision 17
# baseline (speedup 1.0000x reference)
"""Trainium2 Bass kernel for nn_CosineLayer (retrieval_knn).

Computes out = concat(normalize(features) @ normalize(weight).T, threshold_col).

Strategy (tensor/vocab parallel on the 434k concept axis, per sharding hint):
  - Host: L2-normalize features and weight rows (cheap one-pass prep), fold
    normalization + a x64 fp8 scale into the weight, quantize weights to fp8
    e3m4, transpose shards to [K, N_shard] so the contraction dim lands on
    SBUF partitions, pad N to 8*54272.
  - Device (x8 SPMD): pure streaming matmul sim_shard = f_hatT.T @ w_hatT_shard
    (fp16 stationary features x fp8e3 moving weights, fp32 PSUM accumulation
    over K=768 in 6 chunks of 128), DVE copy PSUM->SBUF fp16, DMA out.
    PE-bound at the 1-elem/cell/cycle matmul floor (~275us/core); the e3m4
    weight stream keeps DMA (~195us) comfortably underneath it.
  - Host: concat shard outputs, trim padding, append threshold column.
"""

import os

import numpy as np

import concourse.mybir as mybir
import concourse.tile as tile
from concourse import bacc
from concourse.bass_utils import run_bass_kernel_spmd

N_CORES = 8
B = 256              # feature rows
K = 768              # embedding dim
KC = K // 128        # 6 k-chunks of 128 partitions
N_FULL = 434056      # concept rows
N_SHARD = 54272      # = 106*512; 8*54272 = 434176 (pad 120)
NT = int(os.environ.get("BASS_COSINE_NT", "1024"))   # n-columns per chunk
N_CHUNKS = N_SHARD // NT
OUT_BATCH = int(os.environ.get("BASS_COSINE_OUT_BATCH", "1"))  # chunks per out-DMA
EPS = 1e-8

# weight compute dtype. "e3x": weights stream as fp8 e3m4 (x64 power-of-2
# scale keeps all values in e3m4's normal range; the 1/64 folds into the fp16
# features for free). HW-verified: the plain (non-DoubleRow) matmul path
# upconverts fp8 to FP22, preserving all 4 e3m4 mantissa bits, and mixed
# fp16-stationary x fp8e3-moving is exact vs numpy on quantized values
# (absmax ~1e-6). Measured rel_l2 ~1.3e-2 vs the 2e-2 gate. This halves the
# dominant weight HBM stream vs fp16 (41.7 MB vs 83.3 MB per core), moving
# the kernel from DMA-bound to the PE matmul floor (~275us).
# "fp16x" is the fp16-weight fallback (rel_l2 3.2e-4, ~352us).
MODE = os.environ.get("BASS_COSINE_MODE", "e3x")
OUT_FP16 = MODE in ("fp16x", "e3x")
W_SCALE = 64.0 if MODE == "e3x" else 1.0

_CACHED = {}

_MODES = {
    "fp32r": (mybir.dt.float32r, np.float32),
    "fp32": (mybir.dt.float32, np.float32),
    "fp16": (mybir.dt.float16, np.float16),
    "fp16x": (mybir.dt.float16, np.float16),
    "bf16": (mybir.dt.bfloat16, None),  # np dtype resolved via ml_dtypes
    "e3x": (mybir.dt.float8e3, None),  # np dtype resolved via ml_dtypes
}


def _np_dtype(mode):
    if mode == "bf16":
        import ml_dtypes

        return ml_dtypes.bfloat16
    if mode == "e3x":
        import ml_dtypes

        return ml_dtypes.float8_e3m4
    return _MODES[mode][1]


def _build_bass(mode):
    """Build + compile the single-core program (same NEFF runs on all 8 cores)."""
    assert N_CHUNKS % OUT_BATCH == 0, "OUT_BATCH must divide N_CHUNKS"
    nc = bacc.Bacc("TRN2", target_bir_lowering=False, debug=False,
                   num_devices=N_CORES)
    mmdt = _MODES[mode][0]
    # features stay fp16 when weights are fp8: the accuracy budget is spent
    # on the big weight stream; the tiny feature tile costs nothing in fp16
    fdt = mybir.dt.float16 if mode == "e3x" else mmdt
    fT_d = nc.dram_tensor("fT", [K, B], fdt, kind="ExternalInput").ap()
    wT_d = nc.dram_tensor("wT", [K, N_SHARD], mmdt, kind="ExternalInput").ap()
    odt = mybir.dt.float16 if OUT_FP16 else mybir.dt.float32
    out_d = nc.dram_tensor("out", [B, N_SHARD], odt, kind="ExternalOutput").ap()

    wT_r = wT_d.rearrange("(c p) n -> p c n", p=128)   # [128, KC, N_SHARD]
    fT_r = fT_d.rearrange("(c p) b -> p c b", p=128)   # [128, KC, B]

    with tile.TileContext(nc) as tc:
        with (
            tc.tile_pool(name="fpool", bufs=1) as fpool,
            tc.tile_pool(name="wpool", bufs=4) as wpool,
            tc.tile_pool(name="opool", bufs=3) as opool,
            tc.tile_pool(name="psum", bufs=4, space="PSUM") as psum,
        ):
            fsb = fpool.tile([128, KC, B], fdt)
            # fT rides the ACT ring so it overlaps the first weight chunk's
            # DMA on the SP ring (both gate the first matmul)
            nc.scalar.dma_start(fsb[:], fT_r[:])

            # PE pre-warm: the tensor engine runs at reduced p-states until
            # ~3us of continuous busy, and the real stream can't start until
            # the first weight chunk lands (~10.5us). Fill the wait with
            # dependency-free dummy matmuls on a zeroed tile so the clock
            # ramp happens off the critical path. One long accumulation
            # group: separate start/stop groups on the same PSUM bank
            # serialize with a pipeline drain each (+240ns apiece, measured),
            # while a single group pipelines at 1 matmul per ~25ns. Sized to
            # slightly overshoot the DMA warmup: ending early would idle the
            # PE and restart the ramp; overshoot only delays the first real
            # matmul by the ~25ns tail granularity.
            N_WARM = 245
            dsb = fpool.tile([128, 128], fdt, name="warm", tag="warm")
            nc.vector.memset(dsb[:], 0.0)
            dps = psum.tile([128, 64], mybir.dt.float32, name="ps0", tag="ps0")
            for i in range(N_WARM):
                nc.tensor.matmul(dps[:], dsb[:], dsb[:, 0:64],
                                 start=(i == 0), stop=(i == N_WARM - 1))

            for g in range(N_CHUNKS // OUT_BATCH):
                osb = [
                    opool.tile([128, OUT_BATCH * NT], odt,
                               name=f"osb{b}", tag=f"osb{b}")
                    for b in range(B // 128)
                ]
                for j in range(OUT_BATCH):
                    n = g * OUT_BATCH + j
                    wsb = wpool.tile([128, KC, NT], mmdt)
                    if n == 0:
                        # split the first chunk along c (NOT n: that would
                        # halve the 1KB DMA line size and get descriptor-
                        # bound): the c=0..2 matmuls start ~1.3us earlier on
                        # the first half and cover the second half's arrival
                        nc.sync.dma_start(wsb[:, :KC // 2], wT_r[:, :KC // 2, :NT])
                        nc.sync.dma_start(wsb[:, KC // 2:], wT_r[:, KC // 2:, :NT])
                    else:
                        nc.sync.dma_start(wsb[:], wT_r[:, :, n * NT:(n + 1) * NT])

                    for b in range(B // 128):
                        # h innermost so both h-slices share one LDWEIGHTS
                        # per (b, c) stationary f-tile
                        pss = [
                            psum.tile([128, 512], mybir.dt.float32,
                                      name=f"ps{h}", tag=f"ps{h}")
                            for h in range(NT // 512)
                        ]
                        for c in range(KC):
                            for h in range(NT // 512):
                                nc.tensor.matmul(
                                    pss[h][:],
                                    fsb[:, c, b * 128:(b + 1) * 128],
                                    wsb[:, c, h * 512:(h + 1) * 512],
                                    start=(c == 0),
                                    stop=(c == KC - 1),
                                )
                        for h in range(NT // 512):
                            nc.vector.tensor_copy(
                                osb[b][:, j * NT + h * 512: j * NT + (h + 1) * 512],
                                pss[h][:],
                            )
                # output DMAs ride the ACT HWDGE ring so their sem-waits
                # never block the SP sequencer's weight prefetch (FIFO per
                # ring). Only the LAST group — which sits after the final
                # weight DMA in the SP stream — splits across both rings, so
                # the tail's HBM write receipts (~2.5us each) drain on two
                # parallel chains instead of serializing on one.
                n0 = g * OUT_BATCH * NT
                last = g == N_CHUNKS // OUT_BATCH - 1
                for b in range(B // 128):
                    eng = nc.sync if (last and b == 1) else nc.scalar
                    eng.dma_start(
                        out_d[b * 128:(b + 1) * 128, n0:n0 + OUT_BATCH * NT], osb[b][:]
                    )
    nc.compile()
    return nc


def _run_spmd(nc, in_maps):
    last_exc = None
    for _ in range(3):  # device occasionally needs one recovery execute
        try:
            return run_bass_kernel_spmd(nc, in_maps, core_ids=list(range(N_CORES)))
        except Exception as e:  # noqa: BLE001
            last_exc = e
    raise last_exc


def kernel(features, weight, threshold):
    features = np.asarray(features, dtype=np.float32)
    weight = np.asarray(weight, dtype=np.float32)
    npdt = _np_dtype(MODE)

    f_norm = np.linalg.norm(features, axis=1, keepdims=True)
    f_hat = features / np.maximum(f_norm, EPS)
    # fold the inverse of the fp8 weight scale into the fp16 features so the
    # device matmul needs no rescale (power-of-2: exact)
    f_dt = np.float16 if MODE == "e3x" else npdt
    fT = np.ascontiguousarray(f_hat.T / W_SCALE).astype(f_dt)   # [768, 256]

    w_norm = np.linalg.norm(weight, axis=1, keepdims=True)
    w_inv = (W_SCALE / np.maximum(w_norm, EPS)).astype(np.float32)

    shards = []
    for i in range(N_CORES):
        n0 = i * N_SHARD
        n1 = min(n0 + N_SHARD, N_FULL)
        s = np.zeros((K, N_SHARD), dtype=npdt)
        s[:, : n1 - n0] = (weight[n0:n1].T * w_inv[n0:n1].T).astype(npdt)
        shards.append(s)

    key = ("nc", MODE)
    if key not in _CACHED:
        _CACHED[key] = _build_bass(MODE)
    nc = _CACHED[key]

    in_maps = [{"fT": fT, "wT": shards[i]} for i in range(N_CORES)]
    res = _run_spmd(nc, in_maps)
    _CACHED["last_result"] = res

    out = np.empty((B, N_FULL + 1), dtype=np.float32)
    for i in range(N_CORES):
        n0 = i * N_SHARD
        n1 = min(n0 + N_SHARD, N_FULL)
        out[:, n0:n1] = res.results[i]["out"][:, : n1 - n0].astype(np.float32)
    out[:, N_FULL] = np.float32(threshold)
    return out



# revision 19
# speedup vs baseline: 1.0096x; 1.0096x over previous
"""Trainium2 Bass kernel for nn_CosineLayer (retrieval_knn).

Computes out = concat(normalize(features) @ normalize(weight).T, threshold_col).

Strategy (tensor/vocab parallel on the 434k concept axis, per sharding hint):
  - Host: L2-normalize features and weight rows (cheap one-pass prep), fold
    normalization + a x64 fp8 scale into the weight, quantize weights to fp8
    e3m4, transpose shards to [K, N_shard] so the contraction dim lands on
    SBUF partitions, pad N to 8*54272.
  - Device (x8 SPMD): pure streaming matmul sim_shard = f_hatT.T @ w_hatT_shard
    (fp16 stationary features x fp8e3 moving weights, fp32 PSUM accumulation
    over K=768 in 6 chunks of 128), DVE copy PSUM->SBUF fp16, DMA out.
    PE-bound at the 1-elem/cell/cycle matmul floor (~275us/core); the e3m4
    weight stream keeps DMA (~195us) comfortably underneath it.
  - Host: concat shard outputs, trim padding, append threshold column.
"""

import os

import numpy as np

import concourse.mybir as mybir
import concourse.tile as tile
from concourse import bacc
from concourse.bass_utils import run_bass_kernel_spmd

N_CORES = 8
B = 256              # feature rows
K = 768              # embedding dim
KC = K // 128        # 6 k-chunks of 128 partitions
N_FULL = 434056      # concept rows
N_SHARD = 54272      # = 106*512; 8*54272 = 434176 (pad 120)
NT = int(os.environ.get("BASS_COSINE_NT", "1024"))   # n-columns per chunk
N_CHUNKS = N_SHARD // NT
OUT_BATCH = int(os.environ.get("BASS_COSINE_OUT_BATCH", "1"))  # chunks per out-DMA
EPS = 1e-8

# weight compute dtype. "e3x": weights stream as fp8 e3m4 (x64 power-of-2
# scale keeps all values in e3m4's normal range; the 1/64 folds into the fp16
# features for free). HW-verified: the plain (non-DoubleRow) matmul path
# upconverts fp8 to FP22, preserving all 4 e3m4 mantissa bits, and mixed
# fp16-stationary x fp8e3-moving is exact vs numpy on quantized values
# (absmax ~1e-6). Measured rel_l2 ~1.3e-2 vs the 2e-2 gate. This halves the
# dominant weight HBM stream vs fp16 (41.7 MB vs 83.3 MB per core), moving
# the kernel from DMA-bound to the PE matmul floor (~275us).
# "fp16x" is the fp16-weight fallback (rel_l2 3.2e-4, ~352us).
MODE = os.environ.get("BASS_COSINE_MODE", "e3x")
OUT_FP16 = MODE in ("fp16x", "e3x")
W_SCALE = 64.0 if MODE == "e3x" else 1.0

_CACHED = {}

_MODES = {
    "fp32r": (mybir.dt.float32r, np.float32),
    "fp32": (mybir.dt.float32, np.float32),
    "fp16": (mybir.dt.float16, np.float16),
    "fp16x": (mybir.dt.float16, np.float16),
    "bf16": (mybir.dt.bfloat16, None),  # np dtype resolved via ml_dtypes
    "e3x": (mybir.dt.float8e3, None),  # np dtype resolved via ml_dtypes
}


def _np_dtype(mode):
    if mode == "bf16":
        import ml_dtypes

        return ml_dtypes.bfloat16
    if mode == "e3x":
        import ml_dtypes

        return ml_dtypes.float8_e3m4
    return _MODES[mode][1]


def _build_bass(mode):
    """Build + compile the single-core program (same NEFF runs on all 8 cores)."""
    assert N_CHUNKS % OUT_BATCH == 0, "OUT_BATCH must divide N_CHUNKS"
    nc = bacc.Bacc("TRN2", target_bir_lowering=False, debug=False,
                   num_devices=N_CORES)
    mmdt = _MODES[mode][0]
    # features stay fp16 when weights are fp8: the accuracy budget is spent
    # on the big weight stream; the tiny feature tile costs nothing in fp16
    fdt = mybir.dt.float16 if mode == "e3x" else mmdt
    fT_d = nc.dram_tensor("fT", [K, B], fdt, kind="ExternalInput").ap()
    wT_d = nc.dram_tensor("wT", [K, N_SHARD], mmdt, kind="ExternalInput").ap()
    odt = mybir.dt.float16 if OUT_FP16 else mybir.dt.float32
    out_d = nc.dram_tensor("out", [B, N_SHARD], odt, kind="ExternalOutput").ap()

    wT_r = wT_d.rearrange("(c p) n -> p c n", p=128)   # [128, KC, N_SHARD]
    fT_r = fT_d.rearrange("(c p) b -> p c b", p=128)   # [128, KC, B]

    with tile.TileContext(nc) as tc:
        with (
            tc.tile_pool(name="fpool", bufs=1) as fpool,
            tc.tile_pool(name="wpool", bufs=4) as wpool,
            tc.tile_pool(name="opool", bufs=3) as opool,
            tc.tile_pool(name="psum", bufs=4, space="PSUM") as psum,
        ):
            fsb = fpool.tile([128, KC, B], fdt)
            # fT rides the ACT ring so it overlaps the first weight chunk's
            # DMA on the SP ring (both gate the first matmul)
            nc.scalar.dma_start(fsb[:], fT_r[:])

            # PE pre-warm: the tensor engine runs at reduced p-states until
            # ~3us of continuous busy, and the real stream can't start until
            # the first weight chunk lands (~10.5us). Fill the wait with
            # dependency-free dummy matmuls on a zeroed tile so the clock
            # ramp happens off the critical path. One long accumulation
            # group: separate start/stop groups on the same PSUM bank
            # serialize with a pipeline drain each (+240ns apiece, measured),
            # while a single group pipelines at 1 matmul per ~25ns. Sized to
            # slightly overshoot the DMA warmup: ending early would idle the
            # PE and restart the ramp; overshoot only delays the first real
            # matmul by the ~25ns tail granularity.
            # 150 x ~34ns (ramp-inclusive, measured) ends ~10.8us, right at
            # the first weight chunk's arrival; HW HAM tolerates a small gap
            # (demotion needs >3us idle), so a slight undershoot is safe
            N_WARM = 150
            dsb = fpool.tile([128, 128], fdt, name="warm", tag="warm")
            nc.vector.memset(dsb[:], 0.0)
            dps = psum.tile([128, 64], mybir.dt.float32, name="ps0", tag="ps0")
            for i in range(N_WARM):
                nc.tensor.matmul(dps[:], dsb[:], dsb[:, 0:64],
                                 start=(i == 0), stop=(i == N_WARM - 1))

            for g in range(N_CHUNKS // OUT_BATCH):
                osb = [
                    opool.tile([128, OUT_BATCH * NT], odt,
                               name=f"osb{b}", tag=f"osb{b}")
                    for b in range(B // 128)
                ]
                for j in range(OUT_BATCH):
                    n = g * OUT_BATCH + j
                    wsb = wpool.tile([128, KC, NT], mmdt)
                    nc.sync.dma_start(wsb[:], wT_r[:, :, n * NT:(n + 1) * NT])

                    for b in range(B // 128):
                        # h innermost so both h-slices share one LDWEIGHTS
                        # per (b, c) stationary f-tile
                        pss = [
                            psum.tile([128, 512], mybir.dt.float32,
                                      name=f"ps{h}", tag=f"ps{h}")
                            for h in range(NT // 512)
                        ]
                        for c in range(KC):
                            for h in range(NT // 512):
                                nc.tensor.matmul(
                                    pss[h][:],
                                    fsb[:, c, b * 128:(b + 1) * 128],
                                    wsb[:, c, h * 512:(h + 1) * 512],
                                    start=(c == 0),
                                    stop=(c == KC - 1),
                                )
                        for h in range(NT // 512):
                            nc.vector.tensor_copy(
                                osb[b][:, j * NT + h * 512: j * NT + (h + 1) * 512],
                                pss[h][:],
                            )
                # output DMAs ride the ACT HWDGE ring so their sem-waits
                # never block the SP sequencer's weight prefetch (FIFO per
                # ring). Only the LAST group — which sits after the final
                # weight DMA in the SP stream — splits across both rings, so
                # the tail's HBM write receipts (~2.5us each) drain on two
                # parallel chains instead of serializing on one.
                n0 = g * OUT_BATCH * NT
                last = g == N_CHUNKS // OUT_BATCH - 1
                for b in range(B // 128):
                    eng = nc.sync if (last and b == 1) else nc.scalar
                    eng.dma_start(
                        out_d[b * 128:(b + 1) * 128, n0:n0 + OUT_BATCH * NT], osb[b][:]
                    )
    nc.compile()
    return nc


def _run_spmd(nc, in_maps):
    last_exc = None
    for _ in range(3):  # device occasionally needs one recovery execute
        try:
            return run_bass_kernel_spmd(nc, in_maps, core_ids=list(range(N_CORES)))
        except Exception as e:  # noqa: BLE001
            last_exc = e
    raise last_exc


def kernel(features, weight, threshold):
    features = np.asarray(features, dtype=np.float32)
    weight = np.asarray(weight, dtype=np.float32)
    npdt = _np_dtype(MODE)

    f_norm = np.linalg.norm(features, axis=1, keepdims=True)
    f_hat = features / np.maximum(f_norm, EPS)
    # fold the inverse of the fp8 weight scale into the fp16 features so the
    # device matmul needs no rescale (power-of-2: exact)
    f_dt = np.float16 if MODE == "e3x" else npdt
    fT = np.ascontiguousarray(f_hat.T / W_SCALE).astype(f_dt)   # [768, 256]

    w_norm = np.linalg.norm(weight, axis=1, keepdims=True)
    w_inv = (W_SCALE / np.maximum(w_norm, EPS)).astype(np.float32)

    shards = []
    for i in range(N_CORES):
        n0 = i * N_SHARD
        n1 = min(n0 + N_SHARD, N_FULL)
        s = np.zeros((K, N_SHARD), dtype=npdt)
        s[:, : n1 - n0] = (weight[n0:n1].T * w_inv[n0:n1].T).astype(npdt)
        shards.append(s)

    key = ("nc", MODE)
    if key not in _CACHED:
        _CACHED[key] = _build_bass(MODE)
    nc = _CACHED[key]

    in_maps = [{"fT": fT, "wT": shards[i]} for i in range(N_CORES)]
    res = _run_spmd(nc, in_maps)
    _CACHED["last_result"] = res

    out = np.empty((B, N_FULL + 1), dtype=np.float32)
    for i in range(N_CORES):
        n0 = i * N_SHARD
        n1 = min(n0 + N_SHARD, N_FULL)
        out[:, n0:n1] = res.results[i]["out"][:, : n1 - n0].astype(np.float32)
    out[:, N_FULL] = np.float32(threshold)
    return out



# revision 20
# speedup vs baseline: 1.0100x; 1.0004x over previous
"""Trainium2 Bass kernel for nn_CosineLayer (retrieval_knn).

Computes out = concat(normalize(features) @ normalize(weight).T, threshold_col).

Strategy (tensor/vocab parallel on the 434k concept axis, per sharding hint):
  - Host: L2-normalize features and weight rows (cheap one-pass prep), fold
    normalization + a x64 fp8 scale into the weight, quantize weights to fp8
    e3m4, transpose shards to [K, N_shard] so the contraction dim lands on
    SBUF partitions, pad N to 8*54272.
  - Device (x8 SPMD): pure streaming matmul sim_shard = f_hatT.T @ w_hatT_shard
    (fp16 stationary features x fp8e3 moving weights, fp32 PSUM accumulation
    over K=768 in 6 chunks of 128), DVE copy PSUM->SBUF fp16, DMA out.
    PE-bound at the 1-elem/cell/cycle matmul floor (~275us/core); the e3m4
    weight stream keeps DMA (~195us) comfortably underneath it.
  - Host: concat shard outputs, trim padding, append threshold column.
"""

import os

import numpy as np

import concourse.mybir as mybir
import concourse.tile as tile
from concourse import bacc
from concourse.bass_utils import run_bass_kernel_spmd

N_CORES = 8
B = 256              # feature rows
K = 768              # embedding dim
KC = K // 128        # 6 k-chunks of 128 partitions
N_FULL = 434056      # concept rows
N_SHARD = 54272      # = 106*512; 8*54272 = 434176 (pad 120)
NT = int(os.environ.get("BASS_COSINE_NT", "1024"))   # n-columns per chunk
N_CHUNKS = N_SHARD // NT
OUT_BATCH = int(os.environ.get("BASS_COSINE_OUT_BATCH", "1"))  # chunks per out-DMA
EPS = 1e-8

# weight compute dtype. "e3x": weights stream as fp8 e3m4 (x64 power-of-2
# scale keeps all values in e3m4's normal range; the 1/64 folds into the fp16
# features for free). HW-verified: the plain (non-DoubleRow) matmul path
# upconverts fp8 to FP22, preserving all 4 e3m4 mantissa bits, and mixed
# fp16-stationary x fp8e3-moving is exact vs numpy on quantized values
# (absmax ~1e-6). Measured rel_l2 ~1.3e-2 vs the 2e-2 gate. This halves the
# dominant weight HBM stream vs fp16 (41.7 MB vs 83.3 MB per core), moving
# the kernel from DMA-bound to the PE matmul floor (~275us).
# "fp16x" is the fp16-weight fallback (rel_l2 3.2e-4, ~352us).
MODE = os.environ.get("BASS_COSINE_MODE", "e3x")
OUT_FP16 = MODE in ("fp16x", "e3x")
W_SCALE = 64.0 if MODE == "e3x" else 1.0

_CACHED = {}

_MODES = {
    "fp32r": (mybir.dt.float32r, np.float32),
    "fp32": (mybir.dt.float32, np.float32),
    "fp16": (mybir.dt.float16, np.float16),
    "fp16x": (mybir.dt.float16, np.float16),
    "bf16": (mybir.dt.bfloat16, None),  # np dtype resolved via ml_dtypes
    "e3x": (mybir.dt.float8e3, None),  # np dtype resolved via ml_dtypes
}


def _np_dtype(mode):
    if mode == "bf16":
        import ml_dtypes

        return ml_dtypes.bfloat16
    if mode == "e3x":
        import ml_dtypes

        return ml_dtypes.float8_e3m4
    return _MODES[mode][1]


def _build_bass(mode):
    """Build + compile the single-core program (same NEFF runs on all 8 cores)."""
    assert N_CHUNKS % OUT_BATCH == 0, "OUT_BATCH must divide N_CHUNKS"
    nc = bacc.Bacc("TRN2", target_bir_lowering=False, debug=False,
                   num_devices=N_CORES)
    mmdt = _MODES[mode][0]
    # features stay fp16 when weights are fp8: the accuracy budget is spent
    # on the big weight stream; the tiny feature tile costs nothing in fp16
    fdt = mybir.dt.float16 if mode == "e3x" else mmdt
    fT_d = nc.dram_tensor("fT", [K, B], fdt, kind="ExternalInput").ap()
    wT_d = nc.dram_tensor("wT", [K, N_SHARD], mmdt, kind="ExternalInput").ap()
    odt = mybir.dt.float16 if OUT_FP16 else mybir.dt.float32
    out_d = nc.dram_tensor("out", [B, N_SHARD], odt, kind="ExternalOutput").ap()

    wT_r = wT_d.rearrange("(c p) n -> p c n", p=128)   # [128, KC, N_SHARD]
    fT_r = fT_d.rearrange("(c p) b -> p c b", p=128)   # [128, KC, B]

    with tile.TileContext(nc) as tc:
        with (
            tc.tile_pool(name="fpool", bufs=1) as fpool,
            tc.tile_pool(name="wpool", bufs=4) as wpool,
            tc.tile_pool(name="opool", bufs=3) as opool,
            tc.tile_pool(name="psum", bufs=4, space="PSUM") as psum,
        ):
            fsb = fpool.tile([128, KC, B], fdt)
            # fT rides the ACT ring so it overlaps the first weight chunk's
            # DMA on the SP ring (both gate the first matmul)
            nc.scalar.dma_start(fsb[:], fT_r[:])

            # PE pre-warm: the tensor engine runs at reduced p-states until
            # ~3us of continuous busy, and the real stream can't start until
            # the first weight chunk lands (~10.5us). Fill the wait with
            # dependency-free dummy matmuls on a zeroed tile so the clock
            # ramp happens off the critical path. One long accumulation
            # group: separate start/stop groups on the same PSUM bank
            # serialize with a pipeline drain each (+240ns apiece, measured),
            # while a single group pipelines at 1 matmul per ~25ns. Sized to
            # slightly overshoot the DMA warmup: ending early would idle the
            # PE and restart the ramp; overshoot only delays the first real
            # matmul by the ~25ns tail granularity.
            # measured: 150 dummies (~4.8us busy) is too little for solid HAM
            # promotion (real stream re-ramps), 245 (~8.3us) holds full clock
            # across a 1.7us handoff gap but overshoots the ~10.5us weight
            # arrival. 180 ends ~10.6us: ~6.5us of continuous pre-warm with a
            # near-seamless handoff to the real stream
            N_WARM = 180
            dsb = fpool.tile([128, 128], fdt, name="warm", tag="warm")
            nc.vector.memset(dsb[:], 0.0)
            dps = psum.tile([128, 64], mybir.dt.float32, name="ps0", tag="ps0")
            for i in range(N_WARM):
                nc.tensor.matmul(dps[:], dsb[:], dsb[:, 0:64],
                                 start=(i == 0), stop=(i == N_WARM - 1))

            for g in range(N_CHUNKS // OUT_BATCH):
                osb = [
                    opool.tile([128, OUT_BATCH * NT], odt,
                               name=f"osb{b}", tag=f"osb{b}")
                    for b in range(B // 128)
                ]
                for j in range(OUT_BATCH):
                    n = g * OUT_BATCH + j
                    wsb = wpool.tile([128, KC, NT], mmdt)
                    nc.sync.dma_start(wsb[:], wT_r[:, :, n * NT:(n + 1) * NT])

                    for b in range(B // 128):
                        # h innermost so both h-slices share one LDWEIGHTS
                        # per (b, c) stationary f-tile
                        pss = [
                            psum.tile([128, 512], mybir.dt.float32,
                                      name=f"ps{h}", tag=f"ps{h}")
                            for h in range(NT // 512)
                        ]
                        for c in range(KC):
                            for h in range(NT // 512):
                                nc.tensor.matmul(
                                    pss[h][:],
                                    fsb[:, c, b * 128:(b + 1) * 128],
                                    wsb[:, c, h * 512:(h + 1) * 512],
                                    start=(c == 0),
                                    stop=(c == KC - 1),
                                )
                        for h in range(NT // 512):
                            nc.vector.tensor_copy(
                                osb[b][:, j * NT + h * 512: j * NT + (h + 1) * 512],
                                pss[h][:],
                            )
                # output DMAs ride the ACT HWDGE ring so their sem-waits
                # never block the SP sequencer's weight prefetch (FIFO per
                # ring). Only the LAST group — which sits after the final
                # weight DMA in the SP stream — splits across both rings, so
                # the tail's HBM write receipts (~2.5us each) drain on two
                # parallel chains instead of serializing on one.
                n0 = g * OUT_BATCH * NT
                last = g == N_CHUNKS // OUT_BATCH - 1
                for b in range(B // 128):
                    eng = nc.sync if (last and b == 1) else nc.scalar
                    eng.dma_start(
                        out_d[b * 128:(b + 1) * 128, n0:n0 + OUT_BATCH * NT], osb[b][:]
                    )
    nc.compile()
    return nc


def _run_spmd(nc, in_maps):
    last_exc = None
    for _ in range(3):  # device occasionally needs one recovery execute
        try:
            return run_bass_kernel_spmd(nc, in_maps, core_ids=list(range(N_CORES)))
        except Exception as e:  # noqa: BLE001
            last_exc = e
    raise last_exc


def kernel(features, weight, threshold):
    features = np.asarray(features, dtype=np.float32)
    weight = np.asarray(weight, dtype=np.float32)
    npdt = _np_dtype(MODE)

    f_norm = np.linalg.norm(features, axis=1, keepdims=True)
    f_hat = features / np.maximum(f_norm, EPS)
    # fold the inverse of the fp8 weight scale into the fp16 features so the
    # device matmul needs no rescale (power-of-2: exact)
    f_dt = np.float16 if MODE == "e3x" else npdt
    fT = np.ascontiguousarray(f_hat.T / W_SCALE).astype(f_dt)   # [768, 256]

    w_norm = np.linalg.norm(weight, axis=1, keepdims=True)
    w_inv = (W_SCALE / np.maximum(w_norm, EPS)).astype(np.float32)

    shards = []
    for i in range(N_CORES):
        n0 = i * N_SHARD
        n1 = min(n0 + N_SHARD, N_FULL)
        s = np.zeros((K, N_SHARD), dtype=npdt)
        s[:, : n1 - n0] = (weight[n0:n1].T * w_inv[n0:n1].T).astype(npdt)
        shards.append(s)

    key = ("nc", MODE)
    if key not in _CACHED:
        _CACHED[key] = _build_bass(MODE)
    nc = _CACHED[key]

    in_maps = [{"fT": fT, "wT": shards[i]} for i in range(N_CORES)]
    res = _run_spmd(nc, in_maps)
    _CACHED["last_result"] = res

    out = np.empty((B, N_FULL + 1), dtype=np.float32)
    for i in range(N_CORES):
        n0 = i * N_SHARD
        n1 = min(n0 + N_SHARD, N_FULL)
        out[:, n0:n1] = res.results[i]["out"][:, : n1 - n0].astype(np.float32)
    out[:, N_FULL] = np.float32(threshold)
    return out

